# revision 3
# baseline (speedup 1.0000x reference)
"""Distributed Trainium2 kernel: out = where(x < 0.5, 0.1*x, x).

Elementwise over 67108864 f32 values, data-parallel across 8 NeuronCores
(each core owns a contiguous 8388608-element shard; no communication).

Per core, a raw-bass 3-engine pipeline streams the shard through SBUF in
NBUF ring slots of [128, CHUNK] f32:
  sync  (SP,  HWDGE ring): HBM -> SBUF loads
  vector(DVE):  m = max((x >= 0.5), 0.1) in {0.1, 1.0};  x *= m  (in place)
  scalar(ACT, HWDGE ring): SBUF -> HBM stores

Synchronization uses one semaphore PER RING SLOT for loads and stores.
A single shared DMA semaphore with cumulative thresholds is racy: each
DMA's +16 arrives as 16 independent +1s (one per SDMA engine), and
engine skew lets a sum-based wait pass while one engine still hasn't
delivered its partitions for the awaited DMA. With one semaphore per
slot there is at most one DMA in flight per semaphore, so the
cumulative >= 16*use_count wait is exact. vec_sem increments come from
a single engine in order, so its cumulative +1 threshold is exact.
"""

import os

# Salt the axon cassette/compile cache before jax/the plugin initializes.
# Stale executables from earlier kernel revisions must never be reused.
os.environ.setdefault("AXON_CASSETTE_SALT", "nn-applyltlin-v2-final")

import numpy as np

import concourse.bass as bass
import concourse.mybir as mybir
from concourse.bass_utils import run_bass_kernel_spmd

N_CORES = 8
TOTAL = 67108864
PER_CORE = TOTAL // N_CORES   # 8388608
P = 128
CHUNK = 2048                  # free-dim elements per ring slot (1 MiB tiles)
NT = PER_CORE // (P * CHUNK)  # 32 tiles per core
NBUF = 16                     # ring depth (16 MiB SBUF); must divide NT
LT_W = 0.5
LIN_W = 0.1
VERSION = 7                   # bump on any kernel change: keys cache_bust

_nc_cache = None


def _build() -> bass.Bass:
    import contextlib

    nc = bass.Bass()
    # Dummy input whose shape encodes the kernel version. The axon
    # executable cache can key on module name + operand shapes and reuse a
    # stale NEFF from an older kernel revision; a distinct shape forces a
    # distinct fingerprint.
    nc.declare_dram_parameter(
        "cache_bust", [1, 1, NBUF, VERSION], mybir.dt.float32, isOutput=False
    )
    x_ext = nc.declare_dram_parameter(
        "x", [NT, P, CHUNK], mybir.dt.float32, isOutput=False
    )
    out_ext = nc.declare_dram_parameter(
        "out", [NT, P, CHUNK], mybir.dt.float32, isOutput=True
    )

    with contextlib.ExitStack() as stack:
        block = stack.enter_context(nc.Block())
        ld_sem = [
            stack.enter_context(nc.semaphore(f"ld{b}")) for b in range(NBUF)
        ]
        st_sem = [
            stack.enter_context(nc.semaphore(f"st{b}")) for b in range(NBUF)
        ]
        vec_sem = stack.enter_context(nc.semaphore("vec_sem"))
        xbuf = stack.enter_context(
            nc.sbuf_tensor("xbuf", [P, NBUF * CHUNK], mybir.dt.float32)
        )
        mbuf = stack.enter_context(
            nc.sbuf_tensor("mbuf", [P, CHUNK], mybir.dt.float32)
        )

        def xt(i):
            b = i % NBUF
            return xbuf[:, b * CHUNK : (b + 1) * CHUNK]

        @block.sync
        def _(sync: bass.BassEngine):
            for i in range(NT):
                b = i % NBUF
                if i >= NBUF:
                    # ring-slot reuse: wait for the slot's previous store;
                    # use count so far is i // NBUF
                    sync.wait_ge(st_sem[b], 16 * (i // NBUF))
                sync.dma_start(out=xt(i), in_=x_ext[i]).then_inc(ld_sem[b], 16)

        @block.vector
        def _(vec: bass.BassEngine):
            for i in range(NT):
                b = i % NBUF
                vec.wait_ge(ld_sem[b], 16 * (i // NBUF + 1))
                t = xt(i)
                vec.tensor_scalar(
                    mbuf[:],
                    t,
                    LT_W,
                    LIN_W,
                    mybir.AluOpType.is_ge,
                    mybir.AluOpType.max,
                )
                vec.tensor_tensor(
                    t, t, mbuf[:], mybir.AluOpType.mult
                ).then_inc(vec_sem, 1)

        @block.scalar
        def _(act: bass.BassEngine):
            for i in range(NT):
                b = i % NBUF
                act.wait_ge(vec_sem, i + 1)
                act.dma_start(out=out_ext[i], in_=xt(i)).then_inc(
                    st_sem[b], 16
                )

    return nc


def run(x: np.ndarray, trace: bool = False):
    """Returns (full_output, BassKernelResults)."""
    global _nc_cache
    x = np.ascontiguousarray(np.asarray(x, dtype=np.float32))
    assert x.shape == (TOTAL,), x.shape
    if _nc_cache is None:
        _nc_cache = _build()
    cb = np.zeros((1, 1, NBUF, VERSION), np.float32)
    in_maps = [
        {
            "x": x[c * PER_CORE : (c + 1) * PER_CORE].reshape(NT, P, CHUNK),
            "cache_bust": cb,
        }
        for c in range(N_CORES)
    ]
    res = run_bass_kernel_spmd(
        _nc_cache, in_maps, core_ids=list(range(N_CORES)), trace=trace
    )
    out = np.concatenate(
        [res.results[c]["out"].reshape(-1) for c in range(N_CORES)]
    )
    return out, res


def kernel(x: np.ndarray) -> np.ndarray:
    out, _ = run(x, trace=False)
    return out
